# revision 18
# baseline (speedup 1.0000x reference)
"""Trainium2 Bass kernel for GQA multi-head attention with RoPE (causal).

Sharding (8 NeuronCores): 2-way data parallel over batch x 4-way sequence
parallel within each batch group.
  - core c: batch b = c//4, group rank j = c%4
  - KV: core computes K/V projections (+RoPE on K) for its contiguous 512-row
    chunk of the sequence, then AllGather over the 4-core group.
  - Q: core owns the strided query rows {j, j+4, j+8, ...} of its batch (512
    rows). Striding makes causal attention work identical on every core, so
    one SPMD program serves all 8 cores; causality enters only through a
    single [128, 4, 32] additive band mask (self-similar across kv tiles).
  - Attention in transposed layout (scores^T: kv on partitions). Work is
    organized per GQA cluster (4 q heads sharing one kv head) so one K/V
    stationary serves 4 heads; q processed in 2 pairs of 256 columns with a
    32-col staircase (columns below the causal threshold are never computed).
    Two heads are packed per matmul (one PSUM bank each). Row sums (softmax
    denominators) accumulate into a shared 2-bank PSUM tile via ones-matmuls.
  - Output projection accumulates all 32 heads in PSUM (8 banks, per
    (column-strip, row-block)), writing the output exactly once. The bias bo
    is added on the host after gathering.

All matmuls run in bf16 with fp32 PSUM accumulation.
"""

import os
import sys

sys.path.insert(0, "/opt/trn_rl_repo")
# recover automatically if a previous run left the NeuronCores wedged
os.environ.setdefault("NEURON_RT_RESET_CORES", "1")

import numpy as np
import ml_dtypes

import concourse.bass as bass  # noqa: F401  (registers engine classes)
import concourse.bacc as bacc
import concourse.mybir as mybir
import concourse.tile as tile
from concourse.bass_utils import run_bass_kernel_spmd

BF16 = ml_dtypes.bfloat16

B, S, D = 2, 2048, 4096
H, KVH, DH = 32, 8, 128
ROPE_BASE = 10000.0
NCORES, TPG = 8, 4          # total cores, cores per batch group
KVC = S // TPG              # 512: kv rows per core
TQ = S // TPG               # 512: query rows per core
KC = D // 128               # 32: contraction chunks of 128
KT = S // 128               # 16: kv tiles per batch
NEG = -1.0e9
SCALE = 1.0 / float(np.sqrt(DH))
F32 = mybir.dt.float32
BF = mybir.dt.bfloat16
GROUPS = [[0, 1, 2, 3], [4, 5, 6, 7]]

_NC = None


def _rope(nc, tmp_pool, ps, cos_sb, sin_sb, out_bf):
    """RoPE in [dh, t] layout: out = ps*cos + rotate_half(ps)*sin, bf16 out."""
    T = ps.shape[-1]
    tcos = tmp_pool.tile([128, T], F32, tag="rope_c")
    tsin = tmp_pool.tile([128, T], F32, tag="rope_s")
    nc.vector.tensor_mul(tcos[:], ps[:], cos_sb[:])
    nc.vector.tensor_mul(tsin[0:64, :], ps[64:128, :], sin_sb[0:64, :])
    nc.vector.tensor_mul(tsin[64:128, :], ps[0:64, :], sin_sb[64:128, :])
    nc.vector.tensor_sub(out_bf[0:64, :], tcos[0:64, :], tsin[0:64, :])
    nc.vector.tensor_add(out_bf[64:128, :], tcos[64:128, :], tsin[64:128, :])


def _pair_slots(p):
    """Per q-pair list of (kt, a, N): kv tile, local col start, width."""
    slots = []
    for kt in range(8 * (p + 1)):
        a = max(0, 32 * kt - 256 * p)
        if a >= 256:
            continue
        slots.append((kt, a, 256 - a))
    return slots


def _build(sim_single_core=False):
    nd = 1 if sim_single_core else NCORES
    nc = bacc.Bacc("TRN2", target_bir_lowering=False, debug=False, num_devices=nd)

    xq = nc.declare_dram_parameter("xq", [D, TQ], BF, isOutput=False)
    xkv = nc.declare_dram_parameter("xkv", [D, KVC], BF, isOutput=False)
    wq = nc.declare_dram_parameter("wq", [D, D], BF, isOutput=False)
    wk = nc.declare_dram_parameter("wk", [D, KVH * DH], BF, isOutput=False)
    wv = nc.declare_dram_parameter("wv", [D, KVH * DH], BF, isOutput=False)
    wo = nc.declare_dram_parameter("wo", [D, D], BF, isOutput=False)
    cos_q = nc.declare_dram_parameter("cos_q", [DH, TQ], F32, isOutput=False)
    sin_q = nc.declare_dram_parameter("sin_q", [DH, TQ], F32, isOutput=False)
    cos_kv = nc.declare_dram_parameter("cos_kv", [DH, KVC], F32, isOutput=False)
    sin_kv = nc.declare_dram_parameter("sin_kv", [DH, KVC], F32, isOutput=False)
    dmask = nc.declare_dram_parameter("dmask", [128, 4, 32], F32, isOutput=False)
    out = nc.declare_dram_parameter("out", [TQ, D], F32, isOutput=True)

    k_sh = nc.dram_tensor("k_sh", [KVH, DH, KVC], BF)
    v_sh = nc.dram_tensor("v_sh", [KVC, KVH * DH], BF)
    k_g = nc.dram_tensor("k_g", [TPG, KVH, DH, KVC], BF)
    v_g = nc.dram_tensor("v_g", [TPG, KVC, KVH * DH], BF)

    with tile.TileContext(nc) as tc:
        with (
            tc.tile_pool(name="const", bufs=1) as const,
            tc.tile_pool(name="qtp", bufs=1) as qtp,
        ):
            ones1 = const.tile([128, 1], BF)
            nc.vector.memset(ones1[:], 1.0)
            dm_sb = const.tile([128, 4, 32], F32)
            nc.sync.dma_start(dm_sb[:], dmask[:])
            qT = qtp.tile([128, H, TQ], BF)

            with tc.tile_pool(name="xqp", bufs=1) as xqp:
                # allocated now; DMA issued after the K-proj weight stream so
                # the sync queue serves wk chunks first
                xq_sb = xqp.tile([128, KC, TQ], BF)
                xq_r = xq.rearrange("(kc p) t -> p kc t", p=128)

                with tc.tile_pool(name="xkvp", bufs=1) as xkvp:
                    xkv_sb = xkvp.tile([128, KC, KVC], BF)
                    xkv_r = xkv.rearrange("(kc p) t -> p kc t", p=128)
                    # first half ahead of the wk stream so K proj can start,
                    # remainder issued just behind it
                    for c4 in range(2):
                        nc.sync.dma_start(
                            xkv_sb[:, c4 * 8 : (c4 + 1) * 8],
                            xkv_r[:, c4 * 8 : (c4 + 1) * 8],
                        )
                    cos_kv_sb = xkvp.tile([128, KVC], F32)
                    sin_kv_sb = xkvp.tile([128, KVC], F32)
                    nc.sync.dma_start(cos_kv_sb[:], cos_kv[:])
                    nc.sync.dma_start(sin_kv_sb[:], sin_kv[:])

                    with tc.tile_pool(name="wvB", bufs=1) as wvB:
                        wv_sb = wvB.tile([128, KC, KVH * DH], BF)
                        wv_r = wv.rearrange("(kc p) c -> p kc c", p=128)

                        # ---- Phase A: K projection (streamed 1-head wk
                        #      chunks) + RoPE(K) + AllGather ----
                        with (
                            tc.tile_pool(name="wkA", bufs=3) as wkA,
                            tc.tile_pool(name="ktmp", bufs=2) as ktmp,
                            tc.tile_pool(name="ko", bufs=3) as kop,
                            tc.tile_pool(name="psA", bufs=2, space="PSUM") as psA,
                        ):
                            wk_r = wk.rearrange("(kc p) c -> p kc c", p=128)
                            wk0 = wkA.tile([128, KC, DH], BF, tag="wk_h",
                                           name="wk0")
                            nc.sync.dma_start(wk0[:], wk_r[:, :, 0:DH])
                            for c4 in range(2, 4):
                                nc.sync.dma_start(
                                    xkv_sb[:, c4 * 8 : (c4 + 1) * 8],
                                    xkv_r[:, c4 * 8 : (c4 + 1) * 8],
                                )
                            for h in range(KVH):
                                if h == 0:
                                    wk_h = wk0
                                else:
                                    wk_h = wkA.tile([128, KC, DH], BF,
                                                    tag="wk_h")
                                    nc.sync.dma_start(
                                        wk_h[:],
                                        wk_r[:, :, h * DH : (h + 1) * DH],
                                    )
                                psK = psA.tile([128, KVC], F32, tag="psK")
                                for kc in range(KC):
                                    nc.tensor.matmul(
                                        psK[:],
                                        wk_h[:, kc],
                                        xkv_sb[:, kc],
                                        start=(kc == 0), stop=(kc == KC - 1),
                                    )
                                k_out = kop.tile([128, KVC], BF, tag="k_out")
                                _rope(nc, ktmp, psK, cos_kv_sb, sin_kv_sb, k_out)
                                nc.scalar.dma_start(k_sh[h], k_out[:])

                        if sim_single_core:
                            for g in range(TPG):
                                nc.scalar.dma_start(k_g[g], k_sh[:])
                        else:
                            nc.gpsimd.collective_compute(
                                "AllGather", mybir.AluOpType.bypass,
                                replica_groups=GROUPS, ins=[k_sh[:]], outs=[k_g[:]],
                            )

                        # prefetch V weights and Q inputs (sync queue reaches
                        # these after the wk stream; transfers overlap K proj)
                        for c4 in range(4):
                            nc.sync.dma_start(
                                wv_sb[:, c4 * 8 : (c4 + 1) * 8],
                                wv_r[:, c4 * 8 : (c4 + 1) * 8],
                            )
                        for c4 in range(4):
                            nc.sync.dma_start(
                                xq_sb[:, c4 * 8 : (c4 + 1) * 8],
                                xq_r[:, c4 * 8 : (c4 + 1) * 8],
                            )

                        # ---- Phase B: V projection (xkv-stationary reuse) ----
                        with (
                            tc.tile_pool(name="vo", bufs=3) as vop,
                            tc.tile_pool(name="psB", bufs=2, space="PSUM") as psB,
                        ):
                            for t4 in range(4):
                                psV = psB.tile([128, 2, 512], F32, tag="psV")
                                for kc in range(KC):
                                    for nn in range(2):
                                        nc.tensor.matmul(
                                            psV[:, nn],
                                            xkv_sb[:, kc,
                                                   t4 * 128 : (t4 + 1) * 128],
                                            wv_sb[:, kc,
                                                  nn * 512 : (nn + 1) * 512],
                                            start=(kc == 0), stop=(kc == KC - 1),
                                        )
                                for nn in range(2):
                                    v_out = vop.tile([128, 512], BF, tag="v_out")
                                    nc.vector.tensor_copy(v_out[:], psV[:, nn])
                                    nc.scalar.dma_start(
                                        v_sh[t4 * 128 : (t4 + 1) * 128,
                                             nn * 512 : (nn + 1) * 512],
                                        v_out[:],
                                    )

                        if sim_single_core:
                            for g in range(TPG):
                                nc.scalar.dma_start(v_g[g], v_sh[:])
                        else:
                            nc.gpsimd.collective_compute(
                                "AllGather", mybir.AluOpType.bypass,
                                replica_groups=GROUPS, ins=[v_sh[:]], outs=[v_g[:]],
                            )

                with tc.tile_pool(name="kvp", bufs=1) as kvp:
                    # gathered K/V loads ride the gpsimd queue so they cannot
                    # block weight streaming on the sync queue
                    k_sb = kvp.tile([128, KVH, S], BF)
                    for g in range(TPG):
                        for h in range(KVH):
                            nc.gpsimd.dma_start(
                                k_sb[:, h, g * KVC : (g + 1) * KVC], k_g[g, h]
                            )
                    v_sb = kvp.tile([128, KT, KVH * DH], BF)
                    for kt in range(KT):
                        nc.gpsimd.dma_start(
                            v_sb[:, kt],
                            v_g[kt // 4, (kt % 4) * 128 : (kt % 4 + 1) * 128, :],
                        )

                    # ---- Phase C: Q projection + RoPE ----
                    if True:
                        with (
                            tc.tile_pool(name="wqC", bufs=8) as wqC,
                            tc.tile_pool(name="cqp", bufs=1) as cqp,
                            tc.tile_pool(name="qtmp", bufs=2) as qtmp,
                            tc.tile_pool(name="psC", bufs=3, space="PSUM") as psC,
                        ):
                            cos_q_sb = cqp.tile([128, TQ], F32)
                            sin_q_sb = cqp.tile([128, TQ], F32)
                            nc.sync.dma_start(cos_q_sb[:], cos_q[:])
                            nc.sync.dma_start(sin_q_sb[:], sin_q[:])
                            for h in range(H):
                                wq_h = wqC.tile([128, KC, DH], BF, tag="wq_h")
                                eng = nc.sync if h % 2 == 0 else nc.scalar
                                eng.dma_start(
                                    wq_h[:],
                                    wq[:, h * DH : (h + 1) * DH].rearrange(
                                        "(kc p) c -> p kc c", p=128
                                    ),
                                )
                                ps = psC.tile([128, TQ], F32, tag="psQ")
                                for kc in range(KC):
                                    nc.tensor.matmul(
                                        ps[:],
                                        wq_h[:, kc],
                                        xq_sb[:, kc],
                                        start=(kc == 0), stop=(kc == KC - 1),
                                    )
                                _rope(nc, qtmp, ps, cos_q_sb, sin_q_sb, qT[:, h])

                    with tc.tile_pool(name="attnTp", bufs=1) as attnTp:
                        attnT = attnTp.tile([128, H, TQ], BF)
                        # output-projection weights pool opens before the
                        # attention pools so its first chunks prefetch during
                        # attention
                        with tc.tile_pool(name="woE", bufs=3) as woE:
                            wo_pre = []
                            for i in range(3):
                                wo_c = woE.tile([128, 8, 512], BF, tag="wo_c",
                                                name=f"wo_pre{i}")
                                nc.sync.dma_start(
                                    wo_c[:],
                                    wo[:, 0:512].rearrange(
                                        "(h p) c -> p h c", p=128
                                    )[:, i * 8 : (i + 1) * 8],
                                )
                                wo_pre.append(wo_c)

                            # ---- Phase D: attention per GQA cluster ----
                            with (
                                tc.tile_pool(name="pTp", bufs=3) as pTp,
                                tc.tile_pool(name="pvSp", bufs=2) as pvSp,
                                tc.tile_pool(name="sSp", bufs=1) as sSp,
                                tc.tile_pool(name="bcp", bufs=1) as bcp,
                                tc.tile_pool(name="psSC", bufs=2,
                                             space="PSUM") as psSC,
                                tc.tile_pool(name="psPV", bufs=1,
                                             space="PSUM") as psPV,
                                tc.tile_pool(name="psSR", bufs=1,
                                             space="PSUM") as psSR,
                            ):
                                for c in range(KVH):
                                    for p in range(2):
                                        slots = _pair_slots(p)
                                        K = len(slots)
                                        pv4 = psPV.tile([128, 4, 256], F32,
                                                        tag="pv4")
                                        S2 = psSR.tile([128, 2, 2, 256], F32,
                                                       tag="S2")
                                        stash = [None] * K
                                        for ki in range(K + 2):
                                            if ki < K:
                                                kt, a, N = slots[ki]
                                                sc = psSC.tile(
                                                    [128, 4, 256], F32, tag="sc"
                                                )
                                                ksl = k_sb[:, c,
                                                           kt * 128 :
                                                           (kt + 1) * 128]
                                                if a == 0:
                                                    for u in range(2):
                                                        nc.tensor.matmul(
                                                            sc[:, 2 * u :
                                                               2 * u + 2, :],
                                                            ksl,
                                                            qT[:, 4 * c + 2 * u :
                                                               4 * c + 2 * u + 2,
                                                               256 * p :
                                                               256 * (p + 1)],
                                                            start=True, stop=True,
                                                        )
                                                else:
                                                    for h in range(4):
                                                        nc.tensor.matmul(
                                                            sc[:, h, a:256],
                                                            ksl,
                                                            qT[:, 4 * c + h,
                                                               256 * p + a :
                                                               256 * (p + 1)],
                                                            start=(h % 2 == 0),
                                                            stop=(h % 2 == 1),
                                                        )
                                                if a == 32 * kt - 256 * p:
                                                    nc.vector.tensor_add(
                                                        sc[:, :, a : a + 32],
                                                        sc[:, :, a : a + 32],
                                                        dm_sb[:],
                                                    )
                                                pT = pTp.tile([128, 4, 256], BF,
                                                              tag="pT")
                                                nc.scalar.activation(
                                                    pT[:, :, a:256],
                                                    sc[:, :, a:256],
                                                    mybir.ActivationFunctionType.Exp,
                                                    scale=SCALE,
                                                )
                                                stash[ki] = (kt, a, pT)
                                            if ki >= 2:
                                                kt, a, pT = stash[ki - 2]
                                                first = (ki - 2 == 0)
                                                last = (ki - 2 == K - 1)
                                                vsl = v_sb[:, kt,
                                                           c * DH : (c + 1) * DH]
                                                if a == 0:
                                                    for u in range(2):
                                                        nc.tensor.matmul(
                                                            pv4[:, 2 * u :
                                                                2 * u + 2, :],
                                                            vsl,
                                                            pT[:, 2 * u :
                                                               2 * u + 2, :],
                                                            start=first,
                                                            stop=last,
                                                        )
                                                    for u in range(2):
                                                        nc.tensor.matmul(
                                                            S2[0:1, u], ones1[:],
                                                            pT[:, 2 * u :
                                                               2 * u + 2, :],
                                                            start=first,
                                                            stop=last,
                                                        )
                                                else:
                                                    for h in range(4):
                                                        nc.tensor.matmul(
                                                            pv4[:, h, a:256],
                                                            vsl,
                                                            pT[:, h, a:256],
                                                            start=False,
                                                            stop=(last and
                                                                  h % 2 == 1),
                                                        )
                                                    for h in range(4):
                                                        nc.tensor.matmul(
                                                            S2[0:1, h // 2,
                                                               h % 2, a:256],
                                                            ones1[:],
                                                            pT[:, h, a:256],
                                                            start=False,
                                                            stop=(last and
                                                                  h % 2 == 1),
                                                        )
                                        # drain: normalize pv by row sums
                                        pvS = pvSp.tile([128, 4, 256], F32,
                                                        tag="pvS")
                                        nc.vector.tensor_copy(pvS[:], pv4[:])
                                        sS = sSp.tile([1, 2, 2, 256], F32,
                                                      tag="sS")
                                        nc.vector.tensor_copy(sS[:], S2[0:1])
                                        # reciprocal first (cost is bound by
                                        # per-partition elems, same on [1,1024])
                                        # so the DVE queue never waits on the
                                        # gpsimd broadcast/multiply chain
                                        sSf = sS[:].rearrange(
                                            "o x h n -> o (x h n)"
                                        )
                                        nc.vector.reciprocal_approx_fast(
                                            sSf, sSf
                                        )
                                        bc4 = bcp.tile([128, 4, 256], F32,
                                                       tag="bc4")
                                        for h in range(4):
                                            nc.gpsimd.partition_broadcast(
                                                bc4[:, h],
                                                sS[0:1, h // 2, h % 2],
                                            )
                                        nc.gpsimd.tensor_mul(
                                            attnT[:, 4 * c : 4 * c + 4,
                                                  256 * p : 256 * (p + 1)],
                                            pvS[:], bc4[:],
                                        )

                            # ---- Phase E: output projection (full 32-head
                            #      accumulation per (nn, tq), single write) ----
                            with (
                                tc.tile_pool(name="osb", bufs=3) as osbp,
                                tc.tile_pool(name="psE", bufs=2,
                                             space="PSUM") as psE,
                            ):
                                for nn in range(8):
                                    wo_n = wo[:, nn * 512 :
                                              (nn + 1) * 512].rearrange(
                                        "(h p) c -> p h c", p=128
                                    )
                                    ps_n = [
                                        psE.tile([128, 512], F32, tag=f"eps{tq}",
                                                 name=f"eps{tq}")
                                        for tq in range(4)
                                    ]
                                    for hq in range(4):
                                        if nn == 0 and hq < 3:
                                            wo_c = wo_pre[hq]
                                        else:
                                            wo_c = woE.tile([128, 8, 512], BF,
                                                            tag="wo_c")
                                            nc.sync.dma_start(
                                                wo_c[:],
                                                wo_n[:, hq * 8 : (hq + 1) * 8],
                                            )
                                        for tq in range(4):
                                            for h8 in range(8):
                                                hh = hq * 8 + h8
                                                nc.tensor.matmul(
                                                    ps_n[tq],
                                                    attnT[:, hh, tq * 128 :
                                                          (tq + 1) * 128],
                                                    wo_c[:, h8],
                                                    start=(hh == 0),
                                                    stop=(hh == 31),
                                                )
                                    for tq in range(4):
                                        osb_t = osbp.tile([128, 512], F32,
                                                          tag="osb")
                                        nc.vector.tensor_copy(osb_t[:], ps_n[tq])
                                        nc.sync.dma_start(
                                            out[tq * 128 : (tq + 1) * 128,
                                                nn * 512 : (nn + 1) * 512],
                                            osb_t[:],
                                        )

    nc.compile()
    return nc


def _get_nc():
    global _NC
    if _NC is None:
        _NC = _build()
    return _NC


def _rope_tables_T(positions):
    """cos/sin tables in [DH, T] layout for given absolute positions."""
    inv_freq = 1.0 / (ROPE_BASE ** (np.arange(0, DH, 2, dtype=np.float64) / DH))
    freqs = inv_freq[:, None] * positions[None, :].astype(np.float64)  # (64, T)
    emb = np.concatenate([freqs, freqs], axis=0)  # (128, T)
    return np.cos(emb).astype(np.float32), np.sin(emb).astype(np.float32)


def _band_mask(j):
    """Additive causal band mask [128 kv, 4 heads, 32 q] for group rank j.

    Within kv tile kt, q cols [32kt, 32kt+32) have the causal boundary in
    range: col c (global 32kt+c, position 4(32kt+c)+j) attends kv partition
    kvp (position 128kt+kvp) iff kvp <= 4c+j — independent of kt.
    """
    kvp = np.arange(128)[:, None]
    c = np.arange(32)[None, :]
    m = np.where(kvp <= 4 * c + j, 0.0, NEG).astype(np.float32)
    return np.repeat(m[:, None, :], 4, axis=1)


_LAST_BO = None


def make_in_maps(x, Wq, Wk, Wv, Wo, bo):
    global _LAST_BO
    _LAST_BO = np.asarray(bo, dtype=np.float32)
    wq_bf = Wq.astype(BF16)
    wk_bf = Wk.astype(BF16)
    wv_bf = Wv.astype(BF16)
    wo_bf = Wo.astype(BF16)
    in_maps = []
    for c in range(NCORES):
        b, j = divmod(c, TPG)
        qpos = np.arange(j, S, TPG)
        kvpos = np.arange(j * KVC, (j + 1) * KVC)
        cq, sq = _rope_tables_T(qpos)
        ckv, skv = _rope_tables_T(kvpos)
        in_maps.append({
            "xq": np.ascontiguousarray(x[b, qpos, :].T).astype(BF16),
            "xkv": np.ascontiguousarray(x[b, kvpos, :].T).astype(BF16),
            "wq": wq_bf, "wk": wk_bf, "wv": wv_bf, "wo": wo_bf,
            "cos_q": cq, "sin_q": sq, "cos_kv": ckv, "sin_kv": skv,
            "dmask": _band_mask(j),
        })
    return in_maps


def assemble_output(results):
    out = np.empty((B, S, D), dtype=np.float32)
    for c in range(NCORES):
        b, j = divmod(c, TPG)
        out[b, j::TPG, :] = results[c]["out"]
    if _LAST_BO is not None:
        out += _LAST_BO[None, None, :]
    return out


def kernel(x, Wq, Wk, Wv, Wo, bo):
    nc = _get_nc()
    in_maps = make_in_maps(
        np.asarray(x, dtype=np.float32), np.asarray(Wq), np.asarray(Wk),
        np.asarray(Wv), np.asarray(Wo), np.asarray(bo),
    )
    res = run_bass_kernel_spmd(nc, in_maps, list(range(NCORES)))
    return assemble_output(res.results)


# revision 20
# speedup vs baseline: 1.0104x; 1.0104x over previous
"""Trainium2 Bass kernel for GQA multi-head attention with RoPE (causal).

Sharding (8 NeuronCores): 2-way data parallel over batch x 4-way sequence
parallel within each batch group.
  - core c: batch b = c//4, group rank j = c%4
  - KV: core computes K/V projections (+RoPE on K) for its contiguous 512-row
    chunk of the sequence, then AllGather over the 4-core group.
  - Q: core owns the strided query rows {j, j+4, j+8, ...} of its batch (512
    rows). Striding makes causal attention work identical on every core, so
    one SPMD program serves all 8 cores; causality enters only through a
    single [128, 4, 32] additive band mask (self-similar across kv tiles).
  - Attention in transposed layout (scores^T: kv on partitions). Work is
    organized per GQA cluster (4 q heads sharing one kv head) so one K/V
    stationary serves 4 heads; q processed in 2 pairs of 256 columns with a
    32-col staircase (columns below the causal threshold are never computed).
    Two heads are packed per matmul (one PSUM bank each). Row sums (softmax
    denominators) accumulate into a shared 2-bank PSUM tile via ones-matmuls.
  - Output projection accumulates all 32 heads in PSUM (8 banks, per
    (column-strip, row-block)), writing the output exactly once. The bias bo
    is added on the host after gathering.

All matmuls run in bf16 with fp32 PSUM accumulation.
"""

import os
import sys

sys.path.insert(0, "/opt/trn_rl_repo")
# recover automatically if a previous run left the NeuronCores wedged
os.environ.setdefault("NEURON_RT_RESET_CORES", "1")

import numpy as np
import ml_dtypes

import concourse.bass as bass  # noqa: F401  (registers engine classes)
import concourse.bacc as bacc
import concourse.mybir as mybir
import concourse.tile as tile
from concourse.bass_utils import run_bass_kernel_spmd

BF16 = ml_dtypes.bfloat16

B, S, D = 2, 2048, 4096
H, KVH, DH = 32, 8, 128
ROPE_BASE = 10000.0
NCORES, TPG = 8, 4          # total cores, cores per batch group
KVC = S // TPG              # 512: kv rows per core
TQ = S // TPG               # 512: query rows per core
KC = D // 128               # 32: contraction chunks of 128
KT = S // 128               # 16: kv tiles per batch
NEG = -1.0e9
SCALE = 1.0 / float(np.sqrt(DH))
F32 = mybir.dt.float32
BF = mybir.dt.bfloat16
GROUPS = [[0, 1, 2, 3], [4, 5, 6, 7]]

_NC = None


def _rope(nc, tmp_pool, ps, cos_sb, sin_sb, out_bf):
    """RoPE in [dh, t] layout: out = ps*cos + rotate_half(ps)*sin, bf16 out."""
    T = ps.shape[-1]
    tcos = tmp_pool.tile([128, T], F32, tag="rope_c")
    tsin = tmp_pool.tile([128, T], F32, tag="rope_s")
    nc.vector.tensor_mul(tcos[:], ps[:], cos_sb[:])
    nc.vector.tensor_mul(tsin[0:64, :], ps[64:128, :], sin_sb[0:64, :])
    nc.vector.tensor_mul(tsin[64:128, :], ps[0:64, :], sin_sb[64:128, :])
    nc.vector.tensor_sub(out_bf[0:64, :], tcos[0:64, :], tsin[0:64, :])
    nc.vector.tensor_add(out_bf[64:128, :], tcos[64:128, :], tsin[64:128, :])


def _pair_slots(p):
    """Per q-pair list of (kt, a, N): kv tile, local col start, width."""
    slots = []
    for kt in range(8 * (p + 1)):
        a = max(0, 32 * kt - 256 * p)
        if a >= 256:
            continue
        slots.append((kt, a, 256 - a))
    return slots


def _build(sim_single_core=False):
    nd = 1 if sim_single_core else NCORES
    nc = bacc.Bacc("TRN2", target_bir_lowering=False, debug=False, num_devices=nd)

    xq = nc.declare_dram_parameter("xq", [D, TQ], BF, isOutput=False)
    xkv = nc.declare_dram_parameter("xkv", [D, KVC], BF, isOutput=False)
    wq = nc.declare_dram_parameter("wq", [D, D], BF, isOutput=False)
    wk = nc.declare_dram_parameter("wk", [D, KVH * DH], BF, isOutput=False)
    wv = nc.declare_dram_parameter("wv", [D, KVH * DH], BF, isOutput=False)
    wo = nc.declare_dram_parameter("wo", [D, D], BF, isOutput=False)
    cos_q = nc.declare_dram_parameter("cos_q", [DH, TQ], F32, isOutput=False)
    sin_q = nc.declare_dram_parameter("sin_q", [DH, TQ], F32, isOutput=False)
    cos_kv = nc.declare_dram_parameter("cos_kv", [DH, KVC], F32, isOutput=False)
    sin_kv = nc.declare_dram_parameter("sin_kv", [DH, KVC], F32, isOutput=False)
    dmask = nc.declare_dram_parameter("dmask", [128, 4, 32], F32, isOutput=False)
    out = nc.declare_dram_parameter("out", [TQ, D], F32, isOutput=True)

    k_sh = nc.dram_tensor("k_sh", [KVH, DH, KVC], BF)
    v_sh = nc.dram_tensor("v_sh", [KVC, KVH * DH], BF)
    k_g = nc.dram_tensor("k_g", [TPG, KVH, DH, KVC], BF)
    v_g = nc.dram_tensor("v_g", [TPG, KVC, KVH * DH], BF)

    with tile.TileContext(nc) as tc:
        with (
            tc.tile_pool(name="const", bufs=1) as const,
            tc.tile_pool(name="qtp", bufs=1) as qtp,
        ):
            ones1 = const.tile([128, 1], BF)
            nc.vector.memset(ones1[:], 1.0)
            dm_sb = const.tile([128, 4, 32], F32)
            nc.sync.dma_start(dm_sb[:], dmask[:])
            qT = qtp.tile([128, H, TQ], BF)

            with tc.tile_pool(name="xqp", bufs=1) as xqp:
                # allocated now; DMA issued after the K-proj weight stream so
                # the sync queue serves wk chunks first
                xq_sb = xqp.tile([128, KC, TQ], BF)
                xq_r = xq.rearrange("(kc p) t -> p kc t", p=128)

                with tc.tile_pool(name="xkvp", bufs=1) as xkvp:
                    xkv_sb = xkvp.tile([128, KC, KVC], BF)
                    xkv_r = xkv.rearrange("(kc p) t -> p kc t", p=128)
                    for c4 in range(2):
                        nc.sync.dma_start(
                            xkv_sb[:, c4 * 8 : (c4 + 1) * 8],
                            xkv_r[:, c4 * 8 : (c4 + 1) * 8],
                        )
                    cos_kv_sb = xkvp.tile([128, KVC], F32)
                    sin_kv_sb = xkvp.tile([128, KVC], F32)
                    nc.sync.dma_start(cos_kv_sb[:], cos_kv[:])
                    nc.sync.dma_start(sin_kv_sb[:], sin_kv[:])

                    with tc.tile_pool(name="wvB", bufs=1) as wvB:
                        wv_sb = wvB.tile([128, KC, KVH * DH], BF)
                        wv_r = wv.rearrange("(kc p) c -> p kc c", p=128)

                        # ---- Phase A: K projection (streamed 1-head wk
                        #      chunks) + RoPE(K) + AllGather ----
                        with (
                            tc.tile_pool(name="wkA", bufs=2) as wkA,
                            tc.tile_pool(name="ktmp", bufs=2) as ktmp,
                            tc.tile_pool(name="ko", bufs=3) as kop,
                            tc.tile_pool(name="psA", bufs=2, space="PSUM") as psA,
                        ):
                            wk_r = wk.rearrange("(kc p) c -> p kc c", p=128)
                            wk0 = wkA.tile([128, KC, DH], BF, tag="wk_h",
                                           name="wk0")
                            nc.sync.dma_start(wk0[:], wk_r[:, :, 0:DH])
                            for c4 in range(2, 4):
                                nc.sync.dma_start(
                                    xkv_sb[:, c4 * 8 : (c4 + 1) * 8],
                                    xkv_r[:, c4 * 8 : (c4 + 1) * 8],
                                )
                            for h in range(KVH):
                                if h == 0:
                                    wk_h = wk0
                                else:
                                    wk_h = wkA.tile([128, KC, DH], BF,
                                                    tag="wk_h")
                                    nc.sync.dma_start(
                                        wk_h[:],
                                        wk_r[:, :, h * DH : (h + 1) * DH],
                                    )
                                psK = psA.tile([128, KVC], F32, tag="psK")
                                for kc in range(KC):
                                    nc.tensor.matmul(
                                        psK[:],
                                        wk_h[:, kc],
                                        xkv_sb[:, kc],
                                        start=(kc == 0), stop=(kc == KC - 1),
                                    )
                                k_out = kop.tile([128, KVC], BF, tag="k_out")
                                _rope(nc, ktmp, psK, cos_kv_sb, sin_kv_sb, k_out)
                                nc.scalar.dma_start(k_sh[h], k_out[:])

                        if sim_single_core:
                            for g in range(TPG):
                                nc.scalar.dma_start(k_g[g], k_sh[:])
                        else:
                            nc.gpsimd.collective_compute(
                                "AllGather", mybir.AluOpType.bypass,
                                replica_groups=GROUPS, ins=[k_sh[:]], outs=[k_g[:]],
                            )

                        # prefetch V weights and Q inputs (sync queue reaches
                        # these after the wk stream; transfers overlap K proj)
                        for c4 in range(4):
                            nc.sync.dma_start(
                                wv_sb[:, c4 * 8 : (c4 + 1) * 8],
                                wv_r[:, c4 * 8 : (c4 + 1) * 8],
                            )
                        for c4 in range(4):
                            nc.sync.dma_start(
                                xq_sb[:, c4 * 8 : (c4 + 1) * 8],
                                xq_r[:, c4 * 8 : (c4 + 1) * 8],
                            )

                        # ---- Phase B: V projection (xkv-stationary reuse) ----
                        with (
                            tc.tile_pool(name="vo", bufs=3) as vop,
                            tc.tile_pool(name="psB", bufs=2, space="PSUM") as psB,
                        ):
                            for t4 in range(4):
                                psV = psB.tile([128, 2, 512], F32, tag="psV")
                                for kc in range(KC):
                                    for nn in range(2):
                                        nc.tensor.matmul(
                                            psV[:, nn],
                                            xkv_sb[:, kc,
                                                   t4 * 128 : (t4 + 1) * 128],
                                            wv_sb[:, kc,
                                                  nn * 512 : (nn + 1) * 512],
                                            start=(kc == 0), stop=(kc == KC - 1),
                                        )
                                for nn in range(2):
                                    v_out = vop.tile([128, 512], BF, tag="v_out")
                                    nc.vector.tensor_copy(v_out[:], psV[:, nn])
                                    nc.scalar.dma_start(
                                        v_sh[t4 * 128 : (t4 + 1) * 128,
                                             nn * 512 : (nn + 1) * 512],
                                        v_out[:],
                                    )

                        if sim_single_core:
                            for g in range(TPG):
                                nc.scalar.dma_start(v_g[g], v_sh[:])
                        else:
                            nc.gpsimd.collective_compute(
                                "AllGather", mybir.AluOpType.bypass,
                                replica_groups=GROUPS, ins=[v_sh[:]], outs=[v_g[:]],
                            )

                with tc.tile_pool(name="kvp", bufs=1) as kvp:
                    # gathered K/V loads ride the gpsimd queue so they cannot
                    # block weight streaming on the sync queue
                    k_sb = kvp.tile([128, KVH, S], BF)
                    for g in range(TPG):
                        for h in range(KVH):
                            nc.gpsimd.dma_start(
                                k_sb[:, h, g * KVC : (g + 1) * KVC], k_g[g, h]
                            )
                    v_sb = kvp.tile([128, KT, KVH * DH], BF)
                    for kt in range(KT):
                        nc.gpsimd.dma_start(
                            v_sb[:, kt],
                            v_g[kt // 4, (kt % 4) * 128 : (kt % 4 + 1) * 128, :],
                        )

                    # ---- Phase C: Q projection + RoPE ----
                    if True:
                        with (
                            tc.tile_pool(name="wqC", bufs=7) as wqC,
                            tc.tile_pool(name="cqp", bufs=1) as cqp,
                            tc.tile_pool(name="qtmp", bufs=2) as qtmp,
                            tc.tile_pool(name="psC", bufs=3, space="PSUM") as psC,
                        ):
                            cos_q_sb = cqp.tile([128, TQ], F32)
                            sin_q_sb = cqp.tile([128, TQ], F32)
                            nc.sync.dma_start(cos_q_sb[:], cos_q[:])
                            nc.sync.dma_start(sin_q_sb[:], sin_q[:])
                            for h in range(H):
                                wq_h = wqC.tile([128, KC, DH], BF, tag="wq_h")
                                eng = nc.sync if h % 2 == 0 else nc.scalar
                                eng.dma_start(
                                    wq_h[:],
                                    wq[:, h * DH : (h + 1) * DH].rearrange(
                                        "(kc p) c -> p kc c", p=128
                                    ),
                                )
                                ps = psC.tile([128, TQ], F32, tag="psQ")
                                for kc in range(KC):
                                    nc.tensor.matmul(
                                        ps[:],
                                        wq_h[:, kc],
                                        xq_sb[:, kc],
                                        start=(kc == 0), stop=(kc == KC - 1),
                                    )
                                _rope(nc, qtmp, ps, cos_q_sb, sin_q_sb, qT[:, h])

                    with tc.tile_pool(name="attnTp", bufs=1) as attnTp:
                        attnT = attnTp.tile([128, H, TQ], BF)
                        # output-projection weights pool opens before the
                        # attention pools so its first chunks prefetch during
                        # attention
                        with tc.tile_pool(name="woE", bufs=3) as woE:
                            wo_pre = []
                            for i in range(3):
                                wo_c = woE.tile([128, 8, 512], BF, tag="wo_c",
                                                name=f"wo_pre{i}")
                                nc.sync.dma_start(
                                    wo_c[:],
                                    wo[:, 0:512].rearrange(
                                        "(h p) c -> p h c", p=128
                                    )[:, i * 8 : (i + 1) * 8],
                                )
                                wo_pre.append(wo_c)

                            # ---- Phase D: attention per GQA cluster ----
                            with (
                                tc.tile_pool(name="pTp", bufs=3) as pTp,
                                tc.tile_pool(name="pvSp", bufs=2) as pvSp,
                                tc.tile_pool(name="sSp", bufs=1) as sSp,
                                tc.tile_pool(name="bcp", bufs=1) as bcp,
                                tc.tile_pool(name="psSC", bufs=2,
                                             space="PSUM") as psSC,
                                tc.tile_pool(name="psPV", bufs=1,
                                             space="PSUM") as psPV,
                                tc.tile_pool(name="psSR", bufs=1,
                                             space="PSUM") as psSR,
                            ):
                                for c in range(KVH):
                                    for p in range(2):
                                        slots = _pair_slots(p)
                                        K = len(slots)
                                        pv4 = psPV.tile([128, 4, 256], F32,
                                                        tag="pv4")
                                        S2 = psSR.tile([128, 2, 2, 256], F32,
                                                       tag="S2")
                                        stash = [None] * K
                                        for ki in range(K + 2):
                                            if ki < K:
                                                kt, a, N = slots[ki]
                                                sc = psSC.tile(
                                                    [128, 4, 256], F32, tag="sc"
                                                )
                                                ksl = k_sb[:, c,
                                                           kt * 128 :
                                                           (kt + 1) * 128]
                                                if a == 0:
                                                    for u in range(2):
                                                        nc.tensor.matmul(
                                                            sc[:, 2 * u :
                                                               2 * u + 2, :],
                                                            ksl,
                                                            qT[:, 4 * c + 2 * u :
                                                               4 * c + 2 * u + 2,
                                                               256 * p :
                                                               256 * (p + 1)],
                                                            start=True, stop=True,
                                                        )
                                                else:
                                                    for h in range(4):
                                                        nc.tensor.matmul(
                                                            sc[:, h, a:256],
                                                            ksl,
                                                            qT[:, 4 * c + h,
                                                               256 * p + a :
                                                               256 * (p + 1)],
                                                            start=(h % 2 == 0),
                                                            stop=(h % 2 == 1),
                                                        )
                                                if a == 32 * kt - 256 * p:
                                                    nc.vector.tensor_add(
                                                        sc[:, :, a : a + 32],
                                                        sc[:, :, a : a + 32],
                                                        dm_sb[:],
                                                    )
                                                pT = pTp.tile([128, 4, 256], BF,
                                                              tag="pT")
                                                nc.scalar.activation(
                                                    pT[:, :, a:256],
                                                    sc[:, :, a:256],
                                                    mybir.ActivationFunctionType.Exp,
                                                    scale=SCALE,
                                                )
                                                stash[ki] = (kt, a, pT)
                                            if ki >= 2:
                                                kt, a, pT = stash[ki - 2]
                                                first = (ki - 2 == 0)
                                                last = (ki - 2 == K - 1)
                                                vsl = v_sb[:, kt,
                                                           c * DH : (c + 1) * DH]
                                                if a == 0:
                                                    for u in range(2):
                                                        nc.tensor.matmul(
                                                            pv4[:, 2 * u :
                                                                2 * u + 2, :],
                                                            vsl,
                                                            pT[:, 2 * u :
                                                               2 * u + 2, :],
                                                            start=first,
                                                            stop=last,
                                                        )
                                                    for u in range(2):
                                                        nc.tensor.matmul(
                                                            S2[0:1, u], ones1[:],
                                                            pT[:, 2 * u :
                                                               2 * u + 2, :],
                                                            start=first,
                                                            stop=last,
                                                        )
                                                else:
                                                    for h in range(4):
                                                        nc.tensor.matmul(
                                                            pv4[:, h, a:256],
                                                            vsl,
                                                            pT[:, h, a:256],
                                                            start=False,
                                                            stop=(last and
                                                                  h % 2 == 1),
                                                        )
                                                    for h in range(4):
                                                        nc.tensor.matmul(
                                                            S2[0:1, h // 2,
                                                               h % 2, a:256],
                                                            ones1[:],
                                                            pT[:, h, a:256],
                                                            start=False,
                                                            stop=(last and
                                                                  h % 2 == 1),
                                                        )
                                        # drain: normalize pv by row sums
                                        pvS = pvSp.tile([128, 4, 256], F32,
                                                        tag="pvS")
                                        nc.vector.tensor_copy(pvS[:], pv4[:])
                                        sS = sSp.tile([1, 2, 2, 256], F32,
                                                      tag="sS")
                                        nc.vector.tensor_copy(sS[:], S2[0:1])
                                        # reciprocal first (cost is bound by
                                        # per-partition elems, same on [1,1024])
                                        # so the DVE queue never waits on the
                                        # gpsimd broadcast/multiply chain
                                        sSf = sS[:].rearrange(
                                            "o x h n -> o (x h n)"
                                        )
                                        nc.vector.reciprocal_approx_fast(
                                            sSf, sSf
                                        )
                                        bc4 = bcp.tile([128, 4, 256], F32,
                                                       tag="bc4")
                                        for h in range(4):
                                            nc.gpsimd.partition_broadcast(
                                                bc4[:, h],
                                                sS[0:1, h // 2, h % 2],
                                            )
                                        nc.gpsimd.tensor_mul(
                                            attnT[:, 4 * c : 4 * c + 4,
                                                  256 * p : 256 * (p + 1)],
                                            pvS[:], bc4[:],
                                        )

                            # ---- Phase E: output projection (full 32-head
                            #      accumulation per (nn, tq), single write) ----
                            with (
                                tc.tile_pool(name="osb", bufs=3) as osbp,
                                tc.tile_pool(name="psE", bufs=2,
                                             space="PSUM") as psE,
                            ):
                                for nn in range(8):
                                    wo_n = wo[:, nn * 512 :
                                              (nn + 1) * 512].rearrange(
                                        "(h p) c -> p h c", p=128
                                    )
                                    ps_n = [
                                        psE.tile([128, 512], F32, tag=f"eps{tq}",
                                                 name=f"eps{tq}")
                                        for tq in range(4)
                                    ]
                                    for hq in range(4):
                                        if nn == 0 and hq < 3:
                                            wo_c = wo_pre[hq]
                                        else:
                                            wo_c = woE.tile([128, 8, 512], BF,
                                                            tag="wo_c")
                                            nc.sync.dma_start(
                                                wo_c[:],
                                                wo_n[:, hq * 8 : (hq + 1) * 8],
                                            )
                                        for tq in range(4):
                                            for h8 in range(8):
                                                hh = hq * 8 + h8
                                                nc.tensor.matmul(
                                                    ps_n[tq],
                                                    attnT[:, hh, tq * 128 :
                                                          (tq + 1) * 128],
                                                    wo_c[:, h8],
                                                    start=(hh == 0),
                                                    stop=(hh == 31),
                                                )
                                    for tq in range(4):
                                        osb_t = osbp.tile([128, 512], F32,
                                                          tag="osb")
                                        nc.vector.tensor_copy(osb_t[:], ps_n[tq])
                                        nc.sync.dma_start(
                                            out[tq * 128 : (tq + 1) * 128,
                                                nn * 512 : (nn + 1) * 512],
                                            osb_t[:],
                                        )

    nc.compile()
    return nc


def _get_nc():
    global _NC
    if _NC is None:
        _NC = _build()
    return _NC


def _rope_tables_T(positions):
    """cos/sin tables in [DH, T] layout for given absolute positions."""
    inv_freq = 1.0 / (ROPE_BASE ** (np.arange(0, DH, 2, dtype=np.float64) / DH))
    freqs = inv_freq[:, None] * positions[None, :].astype(np.float64)  # (64, T)
    emb = np.concatenate([freqs, freqs], axis=0)  # (128, T)
    return np.cos(emb).astype(np.float32), np.sin(emb).astype(np.float32)


def _band_mask(j):
    """Additive causal band mask [128 kv, 4 heads, 32 q] for group rank j.

    Within kv tile kt, q cols [32kt, 32kt+32) have the causal boundary in
    range: col c (global 32kt+c, position 4(32kt+c)+j) attends kv partition
    kvp (position 128kt+kvp) iff kvp <= 4c+j — independent of kt.
    """
    kvp = np.arange(128)[:, None]
    c = np.arange(32)[None, :]
    m = np.where(kvp <= 4 * c + j, 0.0, NEG).astype(np.float32)
    return np.repeat(m[:, None, :], 4, axis=1)


_LAST_BO = None


def make_in_maps(x, Wq, Wk, Wv, Wo, bo):
    global _LAST_BO
    _LAST_BO = np.asarray(bo, dtype=np.float32)
    wq_bf = Wq.astype(BF16)
    wk_bf = Wk.astype(BF16)
    wv_bf = Wv.astype(BF16)
    wo_bf = Wo.astype(BF16)
    in_maps = []
    for c in range(NCORES):
        b, j = divmod(c, TPG)
        qpos = np.arange(j, S, TPG)
        kvpos = np.arange(j * KVC, (j + 1) * KVC)
        cq, sq = _rope_tables_T(qpos)
        ckv, skv = _rope_tables_T(kvpos)
        in_maps.append({
            "xq": np.ascontiguousarray(x[b, qpos, :].T).astype(BF16),
            "xkv": np.ascontiguousarray(x[b, kvpos, :].T).astype(BF16),
            "wq": wq_bf, "wk": wk_bf, "wv": wv_bf, "wo": wo_bf,
            "cos_q": cq, "sin_q": sq, "cos_kv": ckv, "sin_kv": skv,
            "dmask": _band_mask(j),
        })
    return in_maps


def assemble_output(results):
    out = np.empty((B, S, D), dtype=np.float32)
    for c in range(NCORES):
        b, j = divmod(c, TPG)
        out[b, j::TPG, :] = results[c]["out"]
    if _LAST_BO is not None:
        out += _LAST_BO[None, None, :]
    return out


def kernel(x, Wq, Wk, Wv, Wo, bo):
    nc = _get_nc()
    in_maps = make_in_maps(
        np.asarray(x, dtype=np.float32), np.asarray(Wq), np.asarray(Wk),
        np.asarray(Wv), np.asarray(Wo), np.asarray(bo),
    )
    res = run_bass_kernel_spmd(nc, in_maps, list(range(NCORES)))
    return assemble_output(res.results)
